# revision 1
# baseline (speedup 1.0000x reference)
"""Trainium2 Bass kernel for Grossberg dynamics (batched gated 17x17 matvecs).

dS/dt = (-DECAY*s + (B-s)*relu(exc) - (C+s)*relu(inh)) / TAU, masked on actions.
Rewritten: dS = Re - 0.1*Ri - s*(0.1875 + Re + Ri),  R = relu(1.25 * total).

Sharding: pure data-parallel over the agent axis across 8 NeuronCores.
Per core: 32768 agents. Macro-tile = 128 partitions x G agents/partition.
Agent a = m*(P*G) + p*G + g (partition-major), so HBM reads are contiguous.

Perf structure (cost model): DVE two-tensor fp16 packed ops run at 0.52
ns/el (2x_1p); TensorReduce has NO fast mode (1x), so the 17-wide row
reduction is a binary tree of tensor_tensor adds. Engine plan:
 - each macro's tree+elementwise chain runs entirely on ONE engine
   (DVE for most macros, GPSIMD/Pool for N_POOL_MACROS of them) to avoid
   cross-engine ping-pong;
 - ACT does unary work (sigmoid/relu) and issues the output stores;
 - two-stage software pipelining: macro m+1's loads/multiply/prep are
   emitted before macro m's tree so every engine always has ready work.
"""

import numpy as np

import concourse.bass as bass
import concourse.bacc as bacc
import concourse.mybir as mybir
from concourse.tile import TileContext
from concourse.bass_utils import run_bass_kernel_spmd

P = 128
N = 17
NCORES = 8
B_TOTAL = 262144
B_CORE = B_TOTAL // NCORES  # 32768
G = 16                      # agents per partition per macro-tile
MACROS = B_CORE // (P * G)  # 16
FW = 40                     # packed small-input row: s(17) pert(17) feas(4) pad(2)
N_POOL_MACROS = 5           # macros whose tree+chain run on GPSIMD
MULT_SPLIT_GS = 1           # agents (of G) of each pool-macro's multiply on GPSIMD
L1_POOL_MACROS = frozenset({15})  # DVE-macros whose tree L1 runs on GPSIMD

FP = mybir.dt.float32
FH = mybir.dt.float16
OP = mybir.AluOpType
AF = mybir.ActivationFunctionType

# Grossberg constants
TAU, DECAY, B_CAP, C_FLOOR = 0.8, 0.15, 1.0, 0.1
LAT_INHIB, DIV_SIGMA = 3.0, 0.3
ALPHA, BETA = 1.5, 0.75
INV_TAU = 1.0 / TAU                  # 1.25
U_BIAS = DECAY * INV_TAU             # 0.1875
LAT_DEN_C = DIV_SIGMA + 1e-6


def build_program():
    nc = bacc.Bacc()
    w_d = nc.dram_tensor("w", [B_CORE, 2, N, N], FH, kind="ExternalInput")
    spf_d = nc.dram_tensor("spf", [B_CORE, FW], FH, kind="ExternalInput")
    out_d = nc.dram_tensor("out", [B_CORE, N], FP, kind="ExternalOutput")

    w_v = w_d[:, :, :, :].rearrange("(m p g) t i j -> m p (g t i j)", p=P, g=G)
    spf_v = spf_d[:, :].rearrange("(m p g) f -> m p (g f)", p=P, g=G)
    out_v = out_d[:, :].rearrange("(m p g) n -> m p (g n)", p=P, g=G)

    R = G * 2 * N          # reduce rows per partition per macro (g, mat, i)
    GN = G * N

    pool_macros = set()
    if N_POOL_MACROS:
        step = MACROS / N_POOL_MACROS
        pool_macros = {int(step * k) for k in range(N_POOL_MACROS)}

    with TileContext(nc) as tc:
        with (
            tc.tile_pool(name="tw", bufs=3) as pool_w,
            tc.tile_pool(name="tp", bufs=4) as pool_p,
            tc.tile_pool(name="ts", bufs=6) as pool_s,
            tc.tile_pool(name="t2", bufs=2) as pool,
        ):
            ctxs = {}

            # per-partition bias constant for the lateral sigmoid trick
            latb = pool.tile([P, 1], FP, tag="latb")
            nc.vector.memset(latb[:], -float(np.log(LAT_DEN_C)))

            def stage_a(m):
                """loads + multiply + everything that needs only spf/prod."""
                E = nc.gpsimd if m in pool_macros else nc.vector
                c = {}
                wbuf = pool_w.tile([P, G * 2 * N * N], FH, tag="wbuf")
                spf = pool_s.tile([P, G * FW], FH, tag="spf")
                half = G * N * N  # half the W row (g = 0..G/2)
                if m == 0:
                    # split the first load so the pipeline ramps up sooner
                    nc.sync.dma_start(out=spf[:], in_=spf_v[m])
                    nc.sync.dma_start(out=wbuf[:, 0:half], in_=w_v[m][:, 0:half])
                    nc.sync.dma_start(out=wbuf[:, half:], in_=w_v[m][:, half:])
                else:
                    # spf first: it is tiny and gates both the prep ops and
                    # the multiply, while wbuf gates only the multiply
                    nc.sync.dma_start(out=spf[:], in_=spf_v[m])
                    nc.sync.dma_start(out=wbuf[:], in_=w_v[m])

                spf3 = spf.rearrange("p (g f) -> p g f", f=FW)
                c["s3"] = s3 = spf3[:, :, 0:N]
                c["pt3"] = pt3 = spf3[:, :, N:2 * N]
                c["fs3"] = spf3[:, :, 2 * N:2 * N + 4]

                # big multiply: prod[g, mat*i, j] = W * s_j  (DVE 2x).
                # For pool-macros, GS agents' worth goes to GPSIMD instead:
                # Pool has headroom and the slice feeds its own (Pool) tree.
                prod = pool_p.tile([P, G * 2 * N * N], FH, tag="prod")
                w4 = wbuf.rearrange("p (g r j) -> p g r j", r=2 * N, j=N)
                p4 = prod.rearrange("p (g r j) -> p g r j", r=2 * N, j=N)
                s4 = s3[:, :, None, :].broadcast_to([P, G, 2 * N, N])
                if m == 0:
                    h = G // 2
                    nc.vector.tensor_tensor(
                        out=p4[:, 0:h], in0=w4[:, 0:h], in1=s4[:, 0:h], op=OP.mult
                    )
                    nc.vector.tensor_tensor(
                        out=p4[:, h:], in0=w4[:, h:], in1=s4[:, h:], op=OP.mult
                    )
                elif m in pool_macros and MULT_SPLIT_GS > 0:
                    # the last pool-macro gets a wider Pool slice: Pool idles
                    # at the schedule tail so the extra slice is free there
                    h = G - (2 if m == 12 else MULT_SPLIT_GS)
                    nc.vector.tensor_tensor(
                        out=p4[:, 0:h], in0=w4[:, 0:h], in1=s4[:, 0:h], op=OP.mult
                    )
                    nc.gpsimd.tensor_tensor(
                        out=p4[:, h:], in0=w4[:, h:], in1=s4[:, h:], op=OP.mult
                    )
                else:
                    nc.vector.tensor_tensor(out=p4, in0=w4, in1=s4, op=OP.mult)
                c["prod"] = prod

                # unary prep on ACT (dep: spf only); pack +/- relus in one
                # tile shaped (g, t, 9) so the env add is a single op later
                rpn = pool.tile([P, 18 * G], FH, tag="rpn")
                rpn4 = rpn.rearrange("p (g t f) -> p g t f", t=2, f=9)
                nc.scalar.activation(rpn4[:, :, 0, :], pt3[:, :, 0:9], AF.Relu)
                nc.scalar.activation(rpn4[:, :, 1, :], pt3[:, :, 0:9], AF.Relu, scale=-1.0)
                c["rpn4"] = rpn4

                # gates input + sigmoids (dep: spf), packed (g, t, 4)
                ve = pool.tile([P, 4 * G], FH, tag="ve")
                ve3 = ve.rearrange("p (g f) -> p g f", f=4)
                E.tensor_tensor(
                    out=ve3, in0=s3[:, :, 13:17], in1=pt3[:, :, 13:17], op=OP.add
                )
                gg = pool.tile([P, 8 * G], FH, tag="gg")
                gg4 = gg.rearrange("p (g t f) -> p g t f", t=2, f=4)
                nc.scalar.activation(gg4[:, :, 0, :], ve3, AF.Sigmoid, scale=ALPHA)
                nc.scalar.activation(gg4[:, :, 1, :], ve3, AF.Sigmoid, scale=-BETA)
                c["gg4"] = gg4

                # lateral inhibition (dep: spf only)
                a01 = pool.tile([P, 2 * G], FH, tag="a01")
                a013 = a01.rearrange("p (g f) -> p g f", f=2)
                E.tensor_tensor(
                    out=a013, in0=s3[:, :, 9:11], in1=s3[:, :, 11:13], op=OP.add
                )
                suma = pool.tile([P, G], FH, tag="suma")
                E.tensor_tensor(
                    out=suma[:, :, None], in0=a013[:, :, 0:1], in1=a013[:, :, 1:2],
                    op=OP.add,
                )
                other = pool.tile([P, 4 * G], FH, tag="other")
                other3 = other.rearrange("p (g f) -> p g f", f=4)
                E.tensor_tensor(
                    out=other3,
                    in0=suma[:, :, None].broadcast_to([P, G, 4]),
                    in1=s3[:, :, 9:13],
                    op=OP.subtract,
                )
                # lat = 3*o/(c+o) = 3*sigmoid(ln(o) - ln(c)) for o >= 0,
                # entirely on the (mostly idle) ACT engine
                lno = pool.tile([P, 4 * G], FH, tag="lno")
                nc.scalar.activation(lno[:], other[:], AF.Ln)
                frac = pool.tile([P, 4 * G], FH, tag="frac")
                nc.scalar.activation(frac[:], lno[:], AF.Sigmoid, bias=latb[:])
                lat = pool.tile([P, 4 * G], FH, tag="lat")
                nc.scalar.activation(lat[:], frac[:], AF.Identity, scale=LAT_INHIB)
                c["lat3"] = lat.rearrange("p (g f) -> p g f", f=4)

                # 17th product column, packed by ACT so the tree's final add
                # stays in DVE 2x mode
                c16 = pool.tile([P, R], FH, tag="c16")
                pr3a = c["prod"].rearrange("p (r j) -> p r j", j=N)
                nc.scalar.copy(out=c16[:], in_=pr3a[:, :, 16])
                c["c16"] = c16
                return c

            def stage_b(m, c):
                """tree reduce + combine, all on one engine.

                Pool-macros use scalar_tensor_tensor: on GPSIMD the cost
                model prices TensorScalarPtr at efficiency 0.6 vs 0.42 for
                TensorTensor (1.43x faster), and the free scalar slot fuses
                the U_BIAS add and the 0.1*Ri scale.
                """
                is_pool = m in pool_macros
                E = nc.gpsimd if is_pool else nc.vector

                def tt(out, in0, in1, op):
                    # NOTE: scalar_tensor_tensor is NOT a legal GPSIMD opcode
                    # on real TRN2 (walrus ISA check rejects it), even though
                    # the cost model prices it; plain tensor_tensor only.
                    E.tensor_tensor(out=out, in0=in0, in1=in1, op=op)

                prod = c["prod"]
                s3, fs3 = c["s3"], c["fs3"]
                pr3 = prod.rearrange("p (r j) -> p r j", j=N)

                l1 = pool.tile([P, R * 8], FH, tag="l1")
                l13 = l1.rearrange("p (r j) -> p r j", j=8)
                if m in L1_POOL_MACROS and not is_pool:
                    # Pool idles at the schedule tail; give it these L1 levels
                    nc.gpsimd.tensor_tensor(
                        out=l13, in0=pr3[:, :, 0:8], in1=pr3[:, :, 8:16], op=OP.add
                    )
                else:
                    tt(l13, pr3[:, :, 0:8], pr3[:, :, 8:16], OP.add)
                l2 = pool.tile([P, R * 4], FH, tag="l2")
                l23 = l2.rearrange("p (r j) -> p r j", j=4)
                tt(l23, l13[:, :, 0:4], l13[:, :, 4:8], OP.add)
                l3 = pool.tile([P, R * 2], FH, tag="l3")
                l33 = l3.rearrange("p (r j) -> p r j", j=2)
                tt(l33, l23[:, :, 0:2], l23[:, :, 2:4], OP.add)
                y0 = pool.tile([P, R], FH, tag="y0")
                tt(y0[:, :, None], l33[:, :, 0:1], l33[:, :, 1:2], OP.add)
                y = pool.tile([P, R], FH, tag="y")
                tt(y[:], y0[:], c["c16"][:], OP.add)
                y4 = y.rearrange("p (g t i) -> p g t i", t=2, i=N)

                # gates on action rows (both matrices in one op)
                tt(y4[:, :, :, 9:13], y4[:, :, :, 9:13], c["gg4"], OP.mult)
                # env drive on need rows (both matrices in one op)
                tt(y4[:, :, :, 0:9], y4[:, :, :, 0:9], c["rpn4"], OP.add)
                # lateral on inhibition action rows
                tt(y4[:, :, 1, 9:13], y4[:, :, 1, 9:13], c["lat3"], OP.add)

                # shunting combine
                te = y4[:, :, 0, :]
                ti = y4[:, :, 1, :]
                Re = pool.tile([P, GN], FH, tag="Re")
                Re3 = Re.rearrange("p (g n) -> p g n", n=N)
                nc.scalar.activation(Re3, te, AF.Relu, scale=INV_TAU)
                Ri = pool.tile([P, GN], FH, tag="Ri")
                Ri3 = Ri.rearrange("p (g n) -> p g n", n=N)
                nc.scalar.activation(Ri3, ti, AF.Relu, scale=INV_TAU)

                t2 = pool.tile([P, GN], FH, tag="t2")
                t1 = pool.tile([P, GN], FH, tag="t1")
                tt(t1[:], Re[:], Ri[:], OP.add)
                if is_pool:
                    nc.gpsimd.tensor_scalar_add(out=t2[:], in0=t1[:], scalar1=U_BIAS)
                else:
                    nc.vector.tensor_scalar_add(out=t2[:], in0=t1[:], scalar1=U_BIAS)
                m1 = pool.tile([P, GN], FH, tag="m1")
                m13 = m1.rearrange("p (g n) -> p g n", n=N)
                t23 = t2.rearrange("p (g n) -> p g n", n=N)
                tt(m13, t23, s3, OP.mult)
                t3 = pool.tile([P, GN], FH, tag="t3")
                Ri01 = pool.tile([P, GN], FH, tag="Ri01")
                Ri013 = Ri01.rearrange("p (g n) -> p g n", n=N)
                nc.scalar.activation(Ri013, ti, AF.Relu, scale=0.1 * INV_TAU)
                tt(t3[:], Re[:], Ri01[:], OP.subtract)
                # combine in fp16 (2x), convert on idle ACT, store from ACT
                ob = pool.tile([P, GN], FH, tag="ob")
                tt(ob[:], t3[:], m1[:], OP.subtract)
                ob3 = ob.rearrange("p (g n) -> p g n", n=N)
                tt(ob3[:, :, 9:13], ob3[:, :, 9:13], fs3, OP.mult)
                obf = pool.tile([P, GN], FP, tag="obf")
                nc.scalar.copy(out=obf[:], in_=ob[:])
                nc.scalar.dma_start(out=out_v[m], in_=obf[:])

            # two-stage software pipeline: A(m+1) before B(m)
            ctxs[0] = stage_a(0)
            for m in range(MACROS):
                if m + 1 < MACROS:
                    ctxs[m + 1] = stage_a(m + 1)
                stage_b(m, ctxs.pop(m))
    if not nc.is_finalized():
        nc.finalize()
    return nc


def make_in_maps(state, w_pos, w_neg, feasibility, perturbation):
    state = np.asarray(state, dtype=np.float32)
    w_pos = np.asarray(w_pos, dtype=np.float32)
    w_neg = np.asarray(w_neg, dtype=np.float32)
    feas = np.asarray(feasibility, dtype=np.float32)
    pert = np.asarray(perturbation, dtype=np.float32)

    B = state.shape[0]
    w_h = np.empty((B, 2, N, N), np.float16)
    w_h[:, 0] = w_pos
    w_h[:, 1] = w_neg
    spf = np.zeros((B, FW), np.float16)
    spf[:, 0:N] = state
    spf[:, N:2 * N] = pert
    spf[:, 2 * N:2 * N + 4] = feas

    in_maps = []
    for c in range(NCORES):
        sl = slice(c * B_CORE, (c + 1) * B_CORE)
        in_maps.append({
            "w": np.ascontiguousarray(w_h[sl]),
            "spf": np.ascontiguousarray(spf[sl]),
        })
    return in_maps


def gather(results):
    return np.concatenate([r["out"] for r in results], axis=0)


def kernel(t=None, state=None, W_pos=None, W_neg=None, feasibility=None, perturbation=None, **_):
    nc = build_program()
    in_maps = make_in_maps(state, W_pos, W_neg, feasibility, perturbation)
    res = run_bass_kernel_spmd(nc, in_maps, list(range(NCORES)))
    return gather(res.results)


if __name__ == "__main__":
    rng = np.random.default_rng(0)
    inputs = {
        "t": rng.standard_normal(1).astype(np.float32),
        "state": rng.random((B_TOTAL, N), dtype=np.float32),
        "W_pos": rng.random((B_TOTAL, N, N), dtype=np.float32),
        "W_neg": rng.random((B_TOTAL, N, N), dtype=np.float32),
        "feasibility": rng.random((B_TOTAL, 4), dtype=np.float32),
        "perturbation": rng.standard_normal((B_TOTAL, N)).astype(np.float32),
    }
    out = kernel(**inputs)
    print(out.shape, out.dtype)

